# revision 87
# baseline (speedup 1.0000x reference)
"""Trainium2 Bass kernel for nn_Attention_62362925138174.

Reference computation (per batch b, with xf = x[b].reshape(C, N), N = H*W):
    q = Wq @ xf            [8,  N]
    k = Wk @ xf            [8,  N]
    v = Wv @ xf            [C,  N]
    score[n, m] = q[:, n] . k[:, m]
    P = softmax(score, axis=n)          (per-column softmax)
    out[c, m] = sum_n v[c, n] P[n, m]
    att = gamma * out + xf

Kernel strategy (8 cores = 4 batches x 2 column-halves of N):
  - Score via the rank-8 q^T k form with 4x PE row tiling: q and k are
    projected on device into partition groups {0,32,64,96} (one matmul with
    zero-padded replicated weights), so each 128-row score tile issues four
    concurrent 512-column matmuls on distinct 32-row PE groups.
  - exp() is split between ScalarE (exact activation) and VectorE using a
    Schraudolph fast-exp: bits16 = trunc(A*s + B) written as int16 is the
    bf16 bit pattern of ~exp(s) (+-3%, zero-mean after calibrating B; the
    softmax normalization cancels the scale, final rel err ~4e-5).
  - V@E accumulates with a ones-row appended to V^T so one PSUM chain gives
    both gamma*(V @ E) (gamma folded into Wv on the host) and colsum(E).
  - 1/colsum via exp(-ln(colsum)) on ScalarE; broadcast on GpSimd; residual
    add uses the exact f32 input.
"""

import numpy as np

import concourse.bass as bass
import concourse.bacc as bacc
import concourse.tile as tile
from concourse import mybir
from concourse.bass_utils import run_bass_kernel_spmd

# Problem shape (hardcoded per contract).
B, C, H, W = 4, 64, 64, 64
N = H * W           # 4096
MHALF = N // 2      # 2048 columns of the score/output handled per core
NT = N // 128       # 32 row-tiles of the score matrix
N_CORES = 8

F32 = mybir.dt.float32
BF16 = mybir.dt.bfloat16
I16 = mybir.dt.int16
I8 = mybir.dt.int8
F8E4 = mybir.dt.float8e4
_NP_BF16 = mybir.dt.np(BF16)

# Schraudolph fast-exp constants for fp8e4 bit patterns (DVE truncates on
# f32->int8 convert; B8/BETA calibrated offline on the actual scores).
# ScalarE's exact exp carries the same 2^(~2/8) scale via its bias so the
# two engines' E tiles stay consistent (softmax cancels common scale).
# Scores arrive from the PE scaled by 64 (fp8 gt/kq are scaled up to stay
# in e4m3 normal range), divided out by the exp input scale.
SSCALE = 64.0
FEXP8_A = 11.541560327111707 / SSCALE   # (2^3 / ln 2) / SSCALE
FEXP8_B = 58.0
EXP_BIAS = 0.17328679513998632          # ln(2)*(58-56)/8
VSCALE = 8.0                   # vaug fp8 values are 8*gamma*v; rcp divides out

_PROGRAM = None


def _scalar_owns_exp(t: int, h: int) -> bool:
    """Split the exp tiles between ScalarE (h=0) and VectorE (h=1) so the
    two engines always run concurrently within an iteration."""
    return h == 0


def _build_program() -> bass.Bass:
    nc = bacc.Bacc()

    # xfp is host-permuted to [xk | other-half]: the n-order only permutes
    # the V@E accumulation, so the kq-projection reads xfp[:, 0:MHALF].
    # Everything the PE touches is fp8 so it never switches weight modes.
    xfp_d = nc.declare_dram_parameter("xfp", [C, N], F8E4, isOutput=False)
    xkf_d = nc.declare_dram_parameter("xkf", [C, MHALF], F32, isOutput=False)
    # packed weights: gt = 64*(Wk^T Wq) zero-padded [64,128] | 64*wv^T*gamma
    # (the q-projection is folded into the score: score = xf^T (G xk))
    wpk_d = nc.declare_dram_parameter("wpk", [C, 192], F8E4, isOutput=False)
    out_d = nc.declare_dram_parameter("out", [C, MHALF], F32, isOutput=True)

    EXP = mybir.ActivationFunctionType.Exp
    LN = mybir.ActivationFunctionType.Ln
    MULT = mybir.AluOpType.mult
    ADD = mybir.AluOpType.add

    from concourse.hw_specs import get_activation_tables

    act_sets = list(get_activation_tables(nc.m.arch))
    nle_id = act_sets.index("natural_log_exp_and_others")

    with TileCtx(nc) as (tc, sing, epool, apool, psS, psO):
        # ---- input loads: few large DMAs (each DMA trigger costs ~600ns of
        # queue time); weights+xkp on the scalar queue feed the k/q
        # projections first, bulk xfp/xkf on the sync queue ----
        wpk_sb = sing.tile([C, 192], F8E4, name="wpk_sb")
        nc.scalar.dma_start(out=wpk_sb, in_=wpk_d[:, :])
        gt_sb = wpk_sb[:, 0:128]
        wv_sb = wpk_sb[:, 128:192]
        # xfp rows 64-127 are zeroed once (gpsimd, idle during the DMA wait)
        # so the main-loop score matmuls can run full-array with no config
        # switch against the DoubleRow V@E matmuls
        xfp_sb = sing.tile([128, N], F8E4, name="xfp_sb")
        nc.gpsimd.memset(xfp_sb[64:128, :], 0.0)
        # first chunk is the kq-projection input: smallest critical path
        nc.sync.dma_start(out=xfp_sb[0:64, 0:MHALF], in_=xfp_d[:, 0:MHALF])
        nc.sync.dma_start(out=xfp_sb[0:64, MHALF:N], in_=xfp_d[:, MHALF:N])
        xkf_sb = sing.tile([C, MHALF], F32, name="xkf_sb")
        # bias constants for the scalar exp (scale-match with fp8 fastexp)
        # and the rcp (divides out VSCALE)
        bexp_sb = sing.tile([128, 1], F32, name="bexp_sb")
        nc.gpsimd.memset(bexp_sb, EXP_BIAS)
        brcp_sb = sing.tile([C, 1], F32, name="brcp_sb")
        nc.gpsimd.memset(brcp_sb, -2.0794415416798357)
        # pin ONE activation table set covering Copy/Exp/Ln: without this the
        # framework reloads tables mid-tail (~2.7us each switch)
        nc.scalar.add_instruction(
            mybir.InstLoadActFuncSet(
                name=nc.get_next_instruction_name(),
                act_func_set_id=nle_id,
                ins=[],
                outs=[],
            )
        )

        # ---- prologue projections (partition-group-0 matmuls while the PE
        # is otherwise DMA-idle): kq = G @ xk, and the V^T tiles ----
        kq_sb = sing.tile([128, MHALF], F8E4, name="kq_sb")
        # vaugT[n, 0:64] = 8*(gamma*Wv @ xf)^T tile, vaugT[n, 64:128] = 1
        # (64 ones-rows make the V@E matmul emit colsum already broadcast
        # across partitions 64-127, so the tail needs no gpsimd broadcast);
        # fp8 with row stride 128 for the DoubleRow weight AP
        vaug_sb = sing.tile([128, NT * 128], F8E4, name="vaug_sb")
        vaug3 = vaug_sb.rearrange("p (t u) -> p t u", u=128)
        nc.vector.memset(vaug3[:, :, 64:128], 1.0)

        def emit_kq_chunk(i):
            kp = psS.tile([128, 512], F32, tag="S", name="kp")
            lo = i * 512
            nc.tensor.matmul(
                kp,
                lhsT=gt_sb[0:64, :],
                rhs=xfp_sb[0:64, lo : lo + 512],
                start=True,
                stop=True,
                tile_position=(0, 0),
            )
            ksl = slice(lo, lo + 512)
            if i % 2 == 0:
                nc.scalar.copy(out=kq_sb[:, ksl], in_=kp)
            else:
                nc.vector.tensor_copy(out=kq_sb[:, ksl], in_=kp)

        def emit_vt_chunk(vv):
            vtp = psS.tile([128, 512], F32, tag="S", name="vtp")
            for i in range(8):
                t = vv * 8 + i
                nc.tensor.matmul(
                    vtp[:, i * 64 : (i + 1) * 64],
                    lhsT=xfp_sb[0:64, t * 128 : (t + 1) * 128],
                    rhs=wv_sb[0:64, :],
                    start=True,
                    stop=True,
                    tile_position=(0, 0),
                )
            dst = vaug3[:, vv * 8 : (vv + 1) * 8, 0:64]
            src = vtp.rearrange("p (i u) -> p i u", u=64)
            if vv % 2 == 0:
                nc.vector.tensor_scalar_mul(dst, src, VSCALE / SSCALE)
            else:
                nc.scalar.mul(dst, src, VSCALE / SSCALE)

        for i in range(4):
            emit_kq_chunk(i)
        for vv in range(4):
            emit_vt_chunk(vv)

        # ---- main loop, software-pipelined: E tiles for a t-PAIR are packed
        # [E_t0 | E_t1] so V@E runs fp8 DoubleRow matmuls (2 contraction
        # tiles per pass); the pair's V@E is emitted after the NEXT pair's
        # first score so the PE never waits on the exp engines ----
        O_ps = psO.tile([128, MHALF], F32, name="O_ps")
        NP2 = NT // 2

        def emit_score_exp(t, pairE):
            # one [128,512] S tile (one PSUM bank) per m-chunk: with 4 pool
            # bufs, score(t+1)'s chunk only waits on exp of the SAME chunk
            # of t, which finished an iteration ago -- no exp-latency stall
            j = t % 2
            for r in range(4):
                S = psS.tile([128, 512], F32, tag="S", name="S_ps")
                # full-array matmul: kq/xfp are zero in rows 64-127, so the
                # 128-partition contraction is exact and the PE keeps one
                # tile config all loop (switches cost ~400ns)
                nc.tensor.matmul(
                    S,
                    lhsT=xfp_sb[:, t * 128 : (t + 1) * 128],
                    rhs=kq_sb[:, r * 512 : (r + 1) * 512],
                    start=True,
                    stop=True,
                )
                h, cc = r // 2, r % 2
                dst = pairE[h][:, j * 1024 + cc * 512 : j * 1024 + (cc + 1) * 512]
                if h == 0:
                    nc.scalar.activation(
                        out=dst, in_=S, func=EXP, scale=1.0 / SSCALE, bias=bexp_sb
                    )
                else:
                    nc.vector.tensor_scalar(dst, S, FEXP8_A, FEXP8_B, MULT, ADD)

        def alloc_pair():
            E0 = epool.tile([128, 2048], F8E4, tag="E", name="E_sb")
            E1 = epool.tile([128, 2048], I8, tag="E", name="Ei_sb")
            return (E0, E1)

        def emit_ve_pair(p, pairE, mid_hook=None):
            va = vaug3[:, 2 * p : 2 * p + 2, 0:128]
            r0 = pairE[0].rearrange("p (j m) -> p j m", j=2)
            r1 = pairE[1].bitcast(F8E4).rearrange("p (j m) -> p j m", j=2)
            for r in range(4):
                h, cc = r // 2, r % 2
                rhs = (r0 if h == 0 else r1)[:, :, cc * 512 : (cc + 1) * 512]
                nc.tensor.matmul(
                    O_ps[:, r * 512 : (r + 1) * 512],
                    lhsT=va,
                    rhs=rhs,
                    start=(p == 0),
                    stop=(p == NP2 - 1),
                    perf_mode=mybir.MatmulPerfMode.DoubleRow,
                )
                if r == 1 and mid_hook is not None:
                    mid_hook()

        pairs = []
        pairE = alloc_pair()
        pairs.append(pairE)
        emit_score_exp(0, pairE)
        for t in range(1, NT):
            if t % 2 == 0:
                pairE = alloc_pair()
                pairs.append(pairE)
            emit_score_exp(t, pairE)
            if t == 8:
                # residual input is only needed by the tail; the memset makes
                # the DMA wait for mid-loop instead of competing for HBM
                # bandwidth with the prologue loads
                nc.vector.memset(xkf_sb[0:1, 0:1], 0.0)
                nc.sync.dma_start(out=xkf_sb, in_=xkf_d[:, :])
            if t % 2 == 0 and t >= 4:
                p0 = t // 2 - 2
                emit_ve_pair(p0, pairs[p0])
        # ---- normalize + residual, store. O_ps rows 64-127 hold colsum
        # already broadcast (the extra ones-rows in vaug). Tile serializes
        # same-PSUM-tile readers in EMISSION order, so emit all colsum reads
        # (LN) before the first O_ps[0:C] read (MUL). LN of m-half 0 is
        # emitted right after the final pair's first two chunks so it
        # overlaps the last two V@E matmuls ----
        lnts, bcss = [], []

        def emit_ln(j):
            sl = slice(j * 1024, (j + 1) * 1024)
            lnt = apool.tile([C, 1024], F32, tag="lnt", name="lnt")
            nc.scalar.activation(out=lnt, in_=O_ps[64:128, sl], func=LN)
            lnts.append(lnt)

        emit_ve_pair(NP2 - 2, pairs[NP2 - 2])
        emit_ve_pair(NP2 - 1, pairs[NP2 - 1])
        emit_ln(0)
        emit_ln(1)
        for j in range(2):
            bcs = apool.tile([C, 1024], BF16, tag="bcs", name="bcs")
            # divide out the VSCALE folded into vaug
            nc.scalar.activation(
                out=bcs, in_=lnts[j], func=EXP, scale=-1.0, bias=brcp_sb
            )
            bcss.append(bcs)
        for j in range(2):
            sl = slice(j * 1024, (j + 1) * 1024)
            tmp = apool.tile([C, 1024], F32, tag="tmp", name="tmp")
            nc.vector.tensor_mul(tmp, O_ps[0:C, sl], bcss[j])
            att = apool.tile([C, 1024], F32, tag="att", name="att")
            nc.vector.tensor_add(att, tmp, xkf_sb[:, sl])
            nc.sync.dma_start(out=out_d[:, sl], in_=att)

    nc.finalize()
    return nc


class TileCtx:
    """TileContext plus the tile pools used by the kernel."""

    def __init__(self, nc: bass.Bass):
        self.nc = nc

    def __enter__(self):
        from contextlib import ExitStack

        self._stack = ExitStack()
        tc = self._stack.enter_context(tile.TileContext(self.nc))
        sing = self._stack.enter_context(tc.tile_pool(name="sing", bufs=1))
        epool = self._stack.enter_context(tc.tile_pool(name="epool", bufs=10))
        apool = self._stack.enter_context(tc.tile_pool(name="apool", bufs=4))
        psS = self._stack.enter_context(tc.tile_pool(name="psS", bufs=4, space="PSUM"))
        psO = self._stack.enter_context(tc.tile_pool(name="psO", bufs=1, space="PSUM"))
        return tc, sing, epool, apool, psS, psO

    def __exit__(self, *exc):
        return self._stack.__exit__(*exc)


def get_program() -> bass.Bass:
    global _PROGRAM
    if _PROGRAM is None:
        _PROGRAM = _build_program()
    return _PROGRAM


def make_in_maps(x, Wq, Wk, Wv, gamma):
    """Shard the full inputs into per-core input maps (host-side prep only:
    reshape/slice, replicated zero-padded weight layouts, cast to bf16)."""
    x = np.ascontiguousarray(np.asarray(x, dtype=np.float32))
    Wq = np.asarray(Wq, dtype=np.float32)
    Wk = np.asarray(Wk, dtype=np.float32)
    Wv = np.asarray(Wv, dtype=np.float32)
    gamma = float(np.asarray(gamma, dtype=np.float32).reshape(()))

    # gt = 64*(Wk^T Wq): kq = gt^T @ xk; score*64 = xf^T kq (the 64 keeps
    # fp8 gt/kq values in e4m3 normal range; exp divides it back out)
    _NP_F8 = mybir.dt.np(mybir.dt.float8e4)
    gt = np.zeros((C, 128), dtype=_NP_F8)
    gt[:, 0:64] = (64.0 * (Wk.T @ Wq)).astype(_NP_F8)
    wvh = (64.0 * gamma * Wv.T).astype(_NP_F8)
    wpk = np.ascontiguousarray(np.concatenate([gt, wvh], axis=1))  # [64, 192]

    in_maps = []
    for core in range(N_CORES):
        b, h = divmod(core, 2)
        xf = x[b].reshape(C, N)
        xk = xf[:, h * MHALF : (h + 1) * MHALF]
        xo = xf[:, (1 - h) * MHALF : (2 - h) * MHALF]
        in_maps.append(
            {
                # n-permuted so this core's key half leads (see kernel docs)
                "xfp": np.ascontiguousarray(
                    np.concatenate([xk, xo], axis=1).astype(_NP_F8)
                ),
                "xkf": np.ascontiguousarray(xk),
                "wpk": wpk,
            }
        )
    return in_maps


def gather(results):
    out = np.empty((B, C, N), dtype=np.float32)
    for core in range(N_CORES):
        b, h = divmod(core, 2)
        out[b][:, h * MHALF : (h + 1) * MHALF] = results[core]["out"]
    return out.reshape(B, C, H, W)


def run(inputs, **spmd_kwargs):
    nc = get_program()
    in_maps = make_in_maps(
        inputs["x"], inputs["Wq"], inputs["Wk"], inputs["Wv"], inputs["gamma"]
    )
    res = run_bass_kernel_spmd(nc, in_maps, core_ids=list(range(N_CORES)), **spmd_kwargs)
    return gather(res.results), res


def kernel(x, Wq, Wk, Wv, gamma):
    out, _ = run({"x": x, "Wq": Wq, "Wk": Wk, "Wv": Wv, "gamma": gamma})
    return out
